# revision 1
# baseline (speedup 1.0000x reference)
"""Trainium2 Bass kernel: 4096x4096 single-channel 3x3 VALID conv + bias.

Sharding: 8-way row-parallel. Core i computes output rows [512*i, 512*i+512)
(core 7: 510 valid rows). Halo handled host-side: each core's input shard is
[514, 4096] (512 rows + 2 halo rows; core 7 zero-padded).

Per core the kernel runs 5 stripes of <=126 output rows. A stripe's 128 input
rows sit on SBUF partitions; for each 512-wide PSUM bank, 3 matmuls (one per
kernel column dj, rhs shifted by dj along the free dim) against 128x126 band
matrices (built host-side from the 3x3 weights) accumulate all 9 taps into
PSUM. ScalarE evacuates PSUM->SBUF fusing the +bias, then stores each half
stripe via its own HWDGE ring as soon as 4 banks are evacuated. Matmuls run
in float32r (full-rate fp32 on PE, ~2e-4 rel err).

Sync notes (hard-won):
- This walrus build allows at most ONE sem wait and ONE sem update per
  instruction; extra waits are standalone engine.wait_ge() instructions.
- An HWDGE dma_start on a compute engine's queue does NOT wait for prior
  compute writes to land; the DMA must be gated by a sem incremented by the
  last producing instruction (finA/finB below), or it reads stale SBUF.
"""

import numpy as np

import concourse.bass as bass
import concourse.mybir as mybir
from concourse.bass_utils import run_bass_kernel_spmd

H = W = 4096
KH = KW = 3
OH = OW = H - KH + 1  # 4094
NCORES = 8
CROWS = 512            # output rows per core (core 7: 510 valid)
IN_ROWS = CROWS + KH - 1  # 514 input rows per core shard
STRIPE = 126           # output rows per full stripe
NBANKS = 8             # PSUM banks; bank b covers output cols [512b, 512b+Nb)
HALF_COL = 2048        # output halves: [0, 2048) and [2048, 4094)

# stripes: (out_row_start, out_rows, in_rows)
STRIPES = []
_r = 0
while _r < CROWS:
    _n = min(STRIPE, CROWS - _r)
    STRIPES.append((_r, _n, _n + KH - 1))
    _r += _n
N_S = len(STRIPES)

_cached = None


def _build():
    nc = bass.Bass()
    x_d = nc.dram_tensor("x", [IN_ROWS, W], mybir.dt.float32r, kind="ExternalInput")
    mb_d = nc.dram_tensor("mb", [128, KW * STRIPE], mybir.dt.float32r, kind="ExternalInput")
    bv_d = nc.dram_tensor("bv", [128, 1], mybir.dt.float32, kind="ExternalInput")
    y_d = nc.dram_tensor("y", [CROWS, OW], mybir.dt.float32, kind="ExternalOutput")

    import contextlib
    with contextlib.ExitStack() as st:
        ec = st.enter_context
        x0 = ec(nc.sbuf_tensor("x0", [128, W], mybir.dt.float32r))
        x1 = ec(nc.sbuf_tensor("x1", [128, W], mybir.dt.float32r))
        x2 = ec(nc.sbuf_tensor("x2", [128, W], mybir.dt.float32r))
        x3 = ec(nc.sbuf_tensor("x3", [128, W], mybir.dt.float32r))
        x4 = ec(nc.sbuf_tensor("x4", [128, W], mybir.dt.float32r))
        y0 = ec(nc.sbuf_tensor("y0", [128, W], mybir.dt.float32))
        y1 = ec(nc.sbuf_tensor("y1", [128, W], mybir.dt.float32))
        mb = ec(nc.sbuf_tensor("mb_sb", [128, KW * STRIPE], mybir.dt.float32r))
        bv = ec(nc.sbuf_tensor("bv_sb", [128, 1], mybir.dt.float32))
        ps = ec(nc.psum_tensor([128, 4096], mybir.dt.float32))
        c_sem = ec(nc.semaphore("c_sem"))
        in0 = ec(nc.semaphore("in0"))
        in0b = ec(nc.semaphore("in0b"))
        in1 = ec(nc.semaphore("in1"))
        in2 = ec(nc.semaphore("in2"))
        in3 = ec(nc.semaphore("in3"))
        in4 = ec(nc.semaphore("in4"))
        pe_sem = ec(nc.semaphore("pe_sem"))
        ev_sem = ec(nc.semaphore("ev_sem"))
        finQ = [ec(nc.semaphore(f"finQ{q}")) for q in range(4)]
        o0 = ec(nc.semaphore("o0"))
        o1 = ec(nc.semaphore("o1"))
        blk = ec(nc.Block())

        xb = [x0, x1, x2, x3, x4]
        yb = [y0, y1]
        ins = [in0, in1, in2, in3, in4]
        outs = [o0, o1]

        def bank_cols(b):
            c0 = 512 * b
            return c0, min(512, OW - c0)

        @blk.sync
        def _(sync):
            sync.dma_start(mb[:], mb_d.ap()).then_inc(c_sem, 16)
            sync.dma_start(bv[:], bv_d.ap()).then_inc(c_sem, 16)
            for s, (r0, orows, irows) in enumerate(STRIPES):
                if s == 0:
                    sync.dma_start(
                        xb[0][0:irows, 0:HALF_COL + 2],
                        x_d.ap()[r0:r0 + irows, 0:HALF_COL + 2],
                    ).then_inc(ins[0], 16)
                    sync.dma_start(
                        xb[0][0:irows, HALF_COL:W],
                        x_d.ap()[r0:r0 + irows, HALF_COL:W],
                    ).then_inc(in0b, 16)
                else:
                    sync.dma_start(
                        xb[s][0:irows, :], x_d.ap()[r0:r0 + irows, :]
                    ).then_inc(ins[s], 16)
            # hold the NEFF open until all outputs are stored
            sync.wait_ge(o0, 64 * ((N_S + 1) // 2))
            sync.wait_ge(o1, 64 * (N_S // 2))

        @blk.tensor
        def _(tensor):
            tensor.wait_ge(c_sem, 32)
            for s, (r0, orows, irows) in enumerate(STRIPES):
                tensor.wait_ge(ins[s], 16)
                xt = xb[s]
                for b in range(NBANKS):
                    c0, nb = bank_cols(b)
                    if s == 0 and b == 4:
                        tensor.wait_ge(in0b, 16)
                    if s >= 1:
                        # previous stripe's bank b must be evacuated
                        if b % 2 == 0:
                            tensor.wait_ge(ev_sem, 4 * (s - 1) + b // 2 + 1)
                        else:
                            tensor.wait_ge(finQ[b // 2], s)
                    mm = None
                    for dj in range(KW):
                        mm = nc.tensor.matmul(
                            ps[0:orows, c0:c0 + nb],
                            mb[0:irows, dj * STRIPE:dj * STRIPE + orows],
                            xt[0:irows, c0 + dj:c0 + dj + nb],
                            start=(dj == 0),
                            stop=(dj == KW - 1),
                        )
                    mm.then_inc(pe_sem, 1)

        @blk.scalar
        def _(scalar):
            scalar.wait_ge(c_sem, 32)
            for s, (r0, orows, irows) in enumerate(STRIPES):
                yt = yb[s % 2]
                if s >= 2:
                    scalar.wait_ge(outs[s % 2], 64 * (s // 2))
                for q in range(4):
                    b = 2 * q
                    c0, nb = bank_cols(b)
                    scalar.wait_ge(pe_sem, NBANKS * s + b + 1)
                    nc.scalar.activation(
                        out=yt[0:orows, c0:c0 + nb],
                        in_=ps[0:orows, c0:c0 + nb],
                        func=mybir.ActivationFunctionType.Identity,
                        bias=bv[0:orows, 0:1],
                        scale=1.0,
                    ).then_inc(ev_sem, 1)

        @blk.vector
        def _(vector):
            vector.wait_ge(c_sem, 32)
            for s, (r0, orows, irows) in enumerate(STRIPES):
                yt = yb[s % 2]
                if s >= 2:
                    vector.wait_ge(outs[s % 2], 64 * (s // 2))
                for q in range(4):
                    b = 2 * q + 1
                    c0, nb = bank_cols(b)
                    vector.wait_ge(pe_sem, NBANKS * s + b + 1)
                    nc.vector.tensor_scalar_add(
                        out=yt[0:orows, c0:c0 + nb],
                        in0=ps[0:orows, c0:c0 + nb],
                        scalar1=bv[0:orows, 0:1],
                    ).then_inc(finQ[q], 1)

        @blk.gpsimd
        def _(gpsimd):
            for s, (r0, orows, irows) in enumerate(STRIPES):
                yt = yb[s % 2]
                for q in range(4):
                    c0 = 1024 * q
                    c1 = min(c0 + 1024, OW)
                    gpsimd.wait_ge(ev_sem, 4 * s + q + 1)
                    gpsimd.wait_ge(finQ[q], s + 1)
                    gpsimd.dma_start(
                        y_d.ap()[r0:r0 + orows, c0:c1],
                        yt[0:orows, c0:c1],
                    ).then_inc(outs[s % 2], 16)

    return nc


def _host_prep(input, weight, bias):
    input = np.ascontiguousarray(input, dtype=np.float32)
    weight = np.asarray(weight, dtype=np.float32)
    bias = np.asarray(bias, dtype=np.float32)

    # band matrices packed side by side: mb[:, dj*126+m] column m of M_dj,
    # M_dj[k, m] = weight[k-m, dj] for 0 <= k-m < KH
    mb = np.zeros((128, KW * STRIPE), dtype=np.float32)
    idx = np.arange(STRIPE)
    for dj in range(KW):
        for di in range(KH):
            mb[idx + di, dj * STRIPE + idx] = weight[di, dj]
    bv = np.full((128, 1), bias[0], dtype=np.float32)

    in_maps = []
    for i in range(NCORES):
        r0 = i * CROWS
        sl = input[r0:r0 + IN_ROWS]
        if sl.shape[0] < IN_ROWS:
            sl = np.concatenate(
                [sl, np.zeros((IN_ROWS - sl.shape[0], W), np.float32)], axis=0
            )
        in_maps.append({"x": np.ascontiguousarray(sl), "mb": mb, "bv": bv})
    return in_maps


def _run(input, weight, bias, **spmd_kwargs):
    global _cached
    if _cached is None:
        _cached = _build()
    in_maps = _host_prep(input, weight, bias)
    res = run_bass_kernel_spmd(
        _cached, in_maps, core_ids=list(range(NCORES)), **spmd_kwargs
    )
    out = np.empty((OH, OW), dtype=np.float32)
    for i in range(NCORES):
        r0 = i * CROWS
        rows = min(CROWS, OH - r0)
        out[r0:r0 + rows] = res.results[i]["y"][:rows]
    return out, res


def kernel(input, weight, bias):
    out, _ = _run(input, weight, bias)
    return out



# revision 5
# speedup vs baseline: 1.3431x; 1.3431x over previous
"""Trainium2 Bass kernel: 4096x4096 single-channel 3x3 VALID conv + bias.

Sharding: 8-way row-parallel. Core i computes output rows [512*i, 512*i+512)
(core 7: 510 valid rows). Halo handled host-side: each core's input shard is
[514, 4096] (512 rows + 2 halo rows; core 7 zero-padded).

Per core the kernel runs 5 stripes of <=126 output rows. A stripe's 128 input
rows sit on SBUF partitions; for each 512-wide PSUM bank, 3 matmuls (one per
kernel column dj, rhs shifted by dj along the free dim) against 128x126 band
matrices (built host-side from the 3x3 weights) accumulate all 9 taps into
PSUM. ScalarE evacuates PSUM->SBUF fusing the +bias, then stores each half
stripe via its own HWDGE ring as soon as 4 banks are evacuated. Matmuls run
in float32r (full-rate fp32 on PE, ~2e-4 rel err).

Sync notes (hard-won):
- This walrus build allows at most ONE sem wait and ONE sem update per
  instruction; extra waits are standalone engine.wait_ge() instructions.
- An HWDGE dma_start on a compute engine's queue does NOT wait for prior
  compute writes to land; the DMA must be gated by a sem incremented by the
  last producing instruction (finA/finB below), or it reads stale SBUF.
"""

import numpy as np

import concourse.bass as bass
import concourse.mybir as mybir
from concourse.bass_utils import run_bass_kernel_spmd

H = W = 4096
KH = KW = 3
OH = OW = H - KH + 1  # 4094
NCORES = 8
CROWS = 512            # output rows per core (core 7: 510 valid)
IN_ROWS = CROWS + KH - 1  # 514 input rows per core shard
STRIPE = 126           # output rows per full stripe
NBANKS = 8             # PSUM banks; bank b covers output cols [512b, 512b+Nb)
HALF_COL = 2048        # output halves: [0, 2048) and [2048, 4094)

# stripes: (out_row_start, out_rows, in_rows)
STRIPES = []
_r = 0
while _r < CROWS:
    _n = min(STRIPE, CROWS - _r)
    STRIPES.append((_r, _n, _n + KH - 1))
    _r += _n
N_S = len(STRIPES)

_cached = None


def _build():
    nc = bass.Bass()
    x_d = nc.dram_tensor("x", [IN_ROWS, W], mybir.dt.bfloat16, kind="ExternalInput")
    mb_d = nc.dram_tensor("mb", [128, KW * STRIPE], mybir.dt.bfloat16, kind="ExternalInput")
    bv_d = nc.dram_tensor("bv", [128, 1], mybir.dt.float32, kind="ExternalInput")
    y_d = nc.dram_tensor("y", [CROWS, OW], mybir.dt.bfloat16, kind="ExternalOutput")

    import contextlib
    with contextlib.ExitStack() as st:
        ec = st.enter_context
        x0 = ec(nc.sbuf_tensor("x0", [128, W], mybir.dt.bfloat16))
        x1 = ec(nc.sbuf_tensor("x1", [128, W], mybir.dt.bfloat16))
        x2 = ec(nc.sbuf_tensor("x2", [128, W], mybir.dt.bfloat16))
        x3 = ec(nc.sbuf_tensor("x3", [128, W], mybir.dt.bfloat16))
        x4 = ec(nc.sbuf_tensor("x4", [128, W], mybir.dt.bfloat16))
        y0 = ec(nc.sbuf_tensor("y0", [128, W], mybir.dt.bfloat16))
        y1 = ec(nc.sbuf_tensor("y1", [128, W], mybir.dt.bfloat16))
        mb = ec(nc.sbuf_tensor("mb_sb", [128, KW * STRIPE], mybir.dt.bfloat16))
        bv = ec(nc.sbuf_tensor("bv_sb", [128, 1], mybir.dt.float32))
        ps = ec(nc.psum_tensor([128, 4096], mybir.dt.float32))
        c_sem = ec(nc.semaphore("c_sem"))
        in0 = ec(nc.semaphore("in0"))
        in0b = ec(nc.semaphore("in0b"))
        in1 = ec(nc.semaphore("in1"))
        in2 = ec(nc.semaphore("in2"))
        in3 = ec(nc.semaphore("in3"))
        in4 = ec(nc.semaphore("in4"))
        pe_sem = ec(nc.semaphore("pe_sem"))
        ev_sem = ec(nc.semaphore("ev_sem"))
        finQ = [ec(nc.semaphore(f"finQ{q}")) for q in range(4)]
        o0 = ec(nc.semaphore("o0"))
        o1 = ec(nc.semaphore("o1"))
        blk = ec(nc.Block())

        xb = [x0, x1, x2, x3, x4]
        yb = [y0, y1]
        ins = [in0, in1, in2, in3, in4]
        outs = [o0, o1]

        def bank_cols(b):
            c0 = 512 * b
            return c0, min(512, OW - c0)

        @blk.sync
        def _(sync):
            sync.dma_start(mb[:], mb_d.ap()).then_inc(c_sem, 16)
            sync.dma_start(bv[:], bv_d.ap()).then_inc(c_sem, 16)
            for s, (r0, orows, irows) in enumerate(STRIPES):
                if s == 0:
                    sync.dma_start(
                        xb[0][0:irows, 0:HALF_COL + 2],
                        x_d.ap()[r0:r0 + irows, 0:HALF_COL + 2],
                    ).then_inc(ins[0], 16)
                    sync.dma_start(
                        xb[0][0:irows, HALF_COL:W],
                        x_d.ap()[r0:r0 + irows, HALF_COL:W],
                    ).then_inc(in0b, 16)
                else:
                    sync.dma_start(
                        xb[s][0:irows, :], x_d.ap()[r0:r0 + irows, :]
                    ).then_inc(ins[s], 16)
            # hold the NEFF open until all outputs are stored
            sync.wait_ge(o0, 64 * ((N_S + 1) // 2))
            sync.wait_ge(o1, 64 * (N_S // 2))

        @blk.tensor
        def _(tensor):
            tensor.wait_ge(c_sem, 32)
            for s, (r0, orows, irows) in enumerate(STRIPES):
                tensor.wait_ge(ins[s], 16)
                xt = xb[s]
                for b in range(NBANKS):
                    c0, nb = bank_cols(b)
                    if s == 0 and b == 4:
                        tensor.wait_ge(in0b, 16)
                    if s >= 1:
                        # previous stripe's bank b must be evacuated
                        if b % 2 == 0:
                            tensor.wait_ge(ev_sem, 4 * (s - 1) + b // 2 + 1)
                        else:
                            tensor.wait_ge(finQ[b // 2], s)
                    mm = None
                    for dj in range(KW):
                        mm = nc.tensor.matmul(
                            ps[0:orows, c0:c0 + nb],
                            mb[0:irows, dj * STRIPE:dj * STRIPE + orows],
                            xt[0:irows, c0 + dj:c0 + dj + nb],
                            start=(dj == 0),
                            stop=(dj == KW - 1),
                        )
                    mm.then_inc(pe_sem, 1)

        @blk.scalar
        def _(scalar):
            scalar.wait_ge(c_sem, 32)
            for s, (r0, orows, irows) in enumerate(STRIPES):
                yt = yb[s % 2]
                if s >= 2:
                    scalar.wait_ge(outs[s % 2], 64 * (s // 2))
                for q in range(4):
                    b = 2 * q
                    c0, nb = bank_cols(b)
                    scalar.wait_ge(pe_sem, NBANKS * s + b + 1)
                    nc.scalar.activation(
                        out=yt[0:orows, c0:c0 + nb],
                        in_=ps[0:orows, c0:c0 + nb],
                        func=mybir.ActivationFunctionType.Identity,
                        bias=bv[0:orows, 0:1],
                        scale=1.0,
                    ).then_inc(ev_sem, 1)

        @blk.vector
        def _(vector):
            vector.wait_ge(c_sem, 32)
            for s, (r0, orows, irows) in enumerate(STRIPES):
                yt = yb[s % 2]
                if s >= 2:
                    vector.wait_ge(outs[s % 2], 64 * (s // 2))
                for q in range(4):
                    b = 2 * q + 1
                    c0, nb = bank_cols(b)
                    vector.wait_ge(pe_sem, NBANKS * s + b + 1)
                    nc.vector.tensor_scalar_add(
                        out=yt[0:orows, c0:c0 + nb],
                        in0=ps[0:orows, c0:c0 + nb],
                        scalar1=bv[0:orows, 0:1],
                    ).then_inc(finQ[q], 1)

        @blk.gpsimd
        def _(gpsimd):
            for s, (r0, orows, irows) in enumerate(STRIPES):
                yt = yb[s % 2]
                for q in range(4):
                    c0 = 1024 * q
                    c1 = min(c0 + 1024, OW)
                    gpsimd.wait_ge(ev_sem, 4 * s + q + 1)
                    gpsimd.wait_ge(finQ[q], s + 1)
                    gpsimd.dma_start(
                        y_d.ap()[r0:r0 + orows, c0:c1],
                        yt[0:orows, c0:c1],
                    ).then_inc(outs[s % 2], 16)

    return nc


def _host_prep(input, weight, bias):
    import ml_dtypes
    bf16 = ml_dtypes.bfloat16
    input = np.ascontiguousarray(np.asarray(input, dtype=np.float32).astype(bf16))
    weight = np.asarray(weight, dtype=np.float32)
    bias = np.asarray(bias, dtype=np.float32)

    # band matrices packed side by side: mb[:, dj*126+m] column m of M_dj,
    # M_dj[k, m] = weight[k-m, dj] for 0 <= k-m < KH
    mb = np.zeros((128, KW * STRIPE), dtype=np.float32)
    idx = np.arange(STRIPE)
    for dj in range(KW):
        for di in range(KH):
            mb[idx + di, dj * STRIPE + idx] = weight[di, dj]
    mb = mb.astype(bf16)
    bv = np.full((128, 1), bias[0], dtype=np.float32)

    in_maps = []
    for i in range(NCORES):
        r0 = i * CROWS
        sl = input[r0:r0 + IN_ROWS]
        if sl.shape[0] < IN_ROWS:
            sl = np.concatenate(
                [sl, np.zeros((IN_ROWS - sl.shape[0], W), bf16)], axis=0
            )
        in_maps.append({"x": np.ascontiguousarray(sl), "mb": mb, "bv": bv})
    return in_maps


def _run(input, weight, bias, **spmd_kwargs):
    global _cached
    if _cached is None:
        _cached = _build()
    in_maps = _host_prep(input, weight, bias)
    res = run_bass_kernel_spmd(
        _cached, in_maps, core_ids=list(range(NCORES)), **spmd_kwargs
    )
    out = np.empty((OH, OW), dtype=np.float32)
    for i in range(NCORES):
        r0 = i * CROWS
        rows = min(CROWS, OH - r0)
        out[r0:r0 + rows] = res.results[i]["y"][:rows].astype(np.float32)
    return out, res


def kernel(input, weight, bias):
    out, _ = _run(input, weight, bias)
    return out



# revision 6
# speedup vs baseline: 1.4496x; 1.0793x over previous
"""Trainium2 Bass kernel: 4096x4096 single-channel 3x3 VALID conv + bias.

Sharding: 2x4 spatial grid over 8 cores. Core c = (rb, cb) = (c//4, c%4)
computes output block rows [2047*rb, 2047*rb+2047) x cols [1024*cb, +1024)
(cb=3: 1022 valid cols). Input shard: [2049, 1026] bf16 (row+col halo,
zero-padded at the right edge for cb=3). All I/O is bf16 (the 2e-2 rel-err
budget dwarfs bf16 rounding, ~5e-3); host converts fp32<->bf16.

Per core: 17 stripes of <=126 output rows. Stripe s input rows sit on the
128 partitions of a dedicated SBUF tile (all 17 x-tiles + 17 y-tiles are
resident; no buffer reuse, so loads have no WAR hazards and are all issued
up-front on the gpsimd queue, whose per-dma_start sequencing cost is 25ns
vs 565ns on sync). For each of the two 512-wide PSUM banks of the stripe's
bank-pair (pair = s%4), 3 matmuls (kernel column dj, rhs shifted by dj)
against 128x126 bf16 band matrices accumulate all 9 taps. ScalarE (even
bank) and VectorE (odd bank) evacuate PSUM->SBUF fusing +bias and the bf16
cast; the sync engine stores each stripe via its own ring.

Sync notes (hard-won, from the fp32 baseline):
- This walrus build allows at most ONE sem wait and ONE sem update per
  instruction; extra waits are standalone engine.wait_ge() instructions.
- An HWDGE dma_start on a compute engine's queue does NOT wait for prior
  compute writes to land; the DMA must be gated by a sem incremented by the
  last producing instruction, or it reads stale SBUF.
- Loads complete nearly in ring order; per-stripe gating uses 4 rotating
  sems (in0..in3) so only loads >=4 apart share a sem (cannot invert).
"""

import numpy as np

import concourse.bass as bass
import concourse.mybir as mybir
from concourse.bass_utils import run_bass_kernel_spmd

H = W = 4096
KH = KW = 3
OH = OW = H - KH + 1   # 4094
NCORES = 8
GRID_R, GRID_C = 2, 4
BROWS = 2047           # output rows per core block (2*2047 = 4094)
BCOLS = 1024           # output cols per core block (cb=3: 1022 valid)
IN_ROWS = BROWS + KH - 1   # 2049
IN_COLS = BCOLS + KW - 1   # 1026
STRIPE = 126           # output rows per full stripe
N_S = 17               # ceil(2047/126): 16 full + one 31-row stripe

# stripes: (out_row_start, out_rows, in_rows)
STRIPES = []
_r = 0
while _r < BROWS:
    _n = min(STRIPE, BROWS - _r)
    STRIPES.append((_r, _n, _n + KH - 1))
    _r += _n
assert len(STRIPES) == N_S

_cached = None


def _build():
    nc = bass.Bass()
    x_d = nc.dram_tensor("x", [IN_ROWS, IN_COLS], mybir.dt.bfloat16, kind="ExternalInput")
    mb_d = nc.dram_tensor("mb", [128, KW * STRIPE], mybir.dt.bfloat16, kind="ExternalInput")
    bv_d = nc.dram_tensor("bv", [128, 1], mybir.dt.float32, kind="ExternalInput")
    y_d = nc.dram_tensor("y", [BROWS, BCOLS], mybir.dt.bfloat16, kind="ExternalOutput")

    import contextlib
    with contextlib.ExitStack() as st:
        ec = st.enter_context
        xb = [ec(nc.sbuf_tensor(f"x{s}", [128, IN_COLS], mybir.dt.bfloat16))
              for s in range(N_S)]
        yb = [ec(nc.sbuf_tensor(f"y{s}", [128, BCOLS], mybir.dt.bfloat16))
              for s in range(N_S)]
        mb = ec(nc.sbuf_tensor("mb_sb", [128, KW * STRIPE], mybir.dt.bfloat16))
        bv = ec(nc.sbuf_tensor("bv_sb", [128, 1], mybir.dt.float32))
        ps = ec(nc.psum_tensor([128, 4096], mybir.dt.float32))
        ldc = ec(nc.semaphore("ldc"))
        ins = [ec(nc.semaphore(f"in{q}")) for q in range(4)]
        pe_sem = ec(nc.semaphore("pe_sem"))
        evA = ec(nc.semaphore("evA"))
        evB = ec(nc.semaphore("evB"))
        st_sem = ec(nc.semaphore("st_sem"))
        blk = ec(nc.Block())

        @blk.gpsimd
        def _(gpsimd):
            gpsimd.dma_start(mb[:], mb_d.ap()).then_inc(ldc, 16)
            gpsimd.dma_start(bv[:], bv_d.ap()).then_inc(ldc, 16)
            for s, (r0, orows, irows) in enumerate(STRIPES):
                gpsimd.dma_start(
                    xb[s][0:irows, :], x_d.ap()[r0:r0 + irows, :]
                ).then_inc(ins[s % 4], 16)

        @blk.tensor
        def _(tensor):
            tensor.wait_ge(ldc, 32)
            # p-state warmup: garbage matmuls on mb while stripe 0 loads
            for _ in range(4):
                nc.tensor.matmul(
                    ps[0:126, 0:KW * STRIPE],
                    mb[0:128, 0:126],
                    mb[0:128, 0:KW * STRIPE],
                    start=True, stop=True,
                )
            for s, (r0, orows, irows) in enumerate(STRIPES):
                p = s % 4
                tensor.wait_ge(ins[p], 16 * (s // 4 + 1))
                if s >= 4:
                    # bank pair p must be evacuated from stripe s-4
                    tensor.wait_ge(evA, s - 3)
                    tensor.wait_ge(evB, s - 3)
                for h in range(2):
                    c0 = 1024 * p + 512 * h
                    mm = None
                    for dj in range(KW):
                        mm = nc.tensor.matmul(
                            ps[0:orows, c0:c0 + 512],
                            mb[0:irows, dj * STRIPE:dj * STRIPE + orows],
                            xb[s][0:irows, 512 * h + dj:512 * h + dj + 512],
                            start=(dj == 0),
                            stop=(dj == KW - 1),
                        )
                    mm.then_inc(pe_sem, 1)

        @blk.scalar
        def _(scalar):
            scalar.wait_ge(ldc, 32)
            for s, (r0, orows, irows) in enumerate(STRIPES):
                p = s % 4
                scalar.wait_ge(pe_sem, 2 * s + 1)
                nc.scalar.activation(
                    out=yb[s][0:orows, 0:512],
                    in_=ps[0:orows, 1024 * p:1024 * p + 512],
                    func=mybir.ActivationFunctionType.Identity,
                    bias=bv[0:orows, 0:1],
                    scale=1.0,
                ).then_inc(evA, 1)

        @blk.vector
        def _(vector):
            vector.wait_ge(ldc, 32)
            for s, (r0, orows, irows) in enumerate(STRIPES):
                p = s % 4
                vector.wait_ge(pe_sem, 2 * s + 2)
                nc.vector.tensor_scalar_add(
                    out=yb[s][0:orows, 512:1024],
                    in0=ps[0:orows, 1024 * p + 512:1024 * p + 1024],
                    scalar1=bv[0:orows, 0:1],
                ).then_inc(evB, 1)

        @blk.sync
        def _(sync):
            for s, (r0, orows, irows) in enumerate(STRIPES):
                sync.wait_ge(evA, s + 1)
                sync.wait_ge(evB, s + 1)
                sync.dma_start(
                    y_d.ap()[r0:r0 + orows, :], yb[s][0:orows, 0:BCOLS]
                ).then_inc(st_sem, 16)
            # hold the NEFF open until all stores land
            sync.wait_ge(st_sem, 16 * N_S)

    return nc


def _host_prep(input, weight, bias):
    import ml_dtypes
    bf16 = ml_dtypes.bfloat16
    input = np.ascontiguousarray(np.asarray(input, dtype=np.float32).astype(bf16))
    weight = np.asarray(weight, dtype=np.float32)
    bias = np.asarray(bias, dtype=np.float32)

    # band matrices packed side by side: mb[:, dj*126+m] column m of M_dj,
    # M_dj[k, m] = weight[k-m, dj] for 0 <= k-m < KH
    mb = np.zeros((128, KW * STRIPE), dtype=np.float32)
    idx = np.arange(STRIPE)
    for dj in range(KW):
        for di in range(KH):
            mb[idx + di, dj * STRIPE + idx] = weight[di, dj]
    mb = mb.astype(bf16)
    bv = np.full((128, 1), bias[0], dtype=np.float32)

    in_maps = []
    for c in range(NCORES):
        rb, cb = c // GRID_C, c % GRID_C
        r0, c0 = rb * BROWS, cb * BCOLS
        sl = input[r0:r0 + IN_ROWS, c0:c0 + IN_COLS]
        if sl.shape[1] < IN_COLS:
            sl = np.concatenate(
                [sl, np.zeros((sl.shape[0], IN_COLS - sl.shape[1]), bf16)], axis=1
            )
        in_maps.append({"x": np.ascontiguousarray(sl), "mb": mb, "bv": bv})
    return in_maps


def _run(input, weight, bias, **spmd_kwargs):
    global _cached
    if _cached is None:
        _cached = _build()
    in_maps = _host_prep(input, weight, bias)
    res = run_bass_kernel_spmd(
        _cached, in_maps, core_ids=list(range(NCORES)), **spmd_kwargs
    )
    out = np.empty((OH, OW), dtype=np.float32)
    for c in range(NCORES):
        rb, cb = c // GRID_C, c % GRID_C
        r0, c0 = rb * BROWS, cb * BCOLS
        cols = min(BCOLS, OW - c0)
        out[r0:r0 + BROWS, c0:c0 + cols] = (
            res.results[c]["y"][:, :cols].astype(np.float32)
        )
    return out, res


def kernel(input, weight, bias):
    out, _ = _run(input, weight, bias)
    return out
